# revision 5
# baseline (speedup 1.0000x reference)
"""MoE (top-2 of 8 experts, D=H=1024, gate MLP 1024->256->8) on 8 Trainium2
NeuronCores, expert-parallel.

Strategy (per the expert-parallel sharding):
  phase 1 (SPMD, data-parallel): each core computes the gate hidden
      h_pre = x_shard @ Wg1 in exact fp32 (1024 tokens/core).
  host: tanh, logits = h @ Wg2, softmax, top-2 routing, MI aux loss, and the
      all-to-all dispatch: gather each expert's tokens (scaled by their gate
      weight -- valid since relu(g*x W) = g*relu(x W) for g>0).
  phase 2 (SPMD, expert-parallel): core e computes
      yT = (relu(x_e @ W_in[e]) @ W_out[e])^T over its capacity-padded token
      set, with float32r matmuls (1 cyc/row on the PE vs 4 for fp32).
  host: scatter-add combine back to [N, D].
"""

import numpy as np
import concourse.mybir as mybir
import concourse.tile as tile
from concourse import bacc
from concourse.bass_utils import run_bass_kernel_spmd

F32 = mybir.dt.float32
F32R = mybir.dt.float32r
AF = mybir.ActivationFunctionType

TOP_K = 2
EPS = 1e-9
B, S, D, H, E, G = 4, 2048, 1024, 1024, 8, 256
N = B * S
NCORES = 8
NSHARD = N // NCORES

# test.py introspection: phase name -> exec_time_ns of the last kernel() call
# (populated only when BASS_TRACE=1 enables NTFF profiling).
last_exec_ns = {}

_kernel_cache = {}


def _build_gate_kernel():
    """Per-core: hT[256, NSHARD] = (x_shard @ Wg1)^T, f32r matmuls (~1e-4 abs
    noise; borderline top-2 routing decisions are re-verified exactly on host)."""
    nc = bacc.Bacc("TRN2", target_bir_lowering=False, debug=False)
    xT = nc.dram_tensor("xT", [D, NSHARD], F32R, kind="ExternalInput").ap()
    wg1 = nc.dram_tensor("wg1", [D, G], F32R, kind="ExternalInput").ap()
    hT = nc.dram_tensor("hT", [G, NSHARD], F32, kind="ExternalOutput").ap()
    n_ct = NSHARD // 512
    n_dt = D // 128
    n_gt = G // 128
    with tile.TileContext(nc) as tc:
        with (
            tc.tile_pool(name="wpool", bufs=1) as wpool,
            tc.tile_pool(name="xpool", bufs=2) as xpool,
            tc.tile_pool(name="opool", bufs=4) as opool,
            tc.tile_pool(name="ps", bufs=4, space="PSUM") as ps,
        ):
            wg1ts = [wpool.tile([128, G], F32R, tag=f"wg1_{dt}", name=f"wg1_{dt}")
                     for dt in range(n_dt)]
            xts0 = [xpool.tile([128, 512], F32R, tag=f"xt{dt}", name=f"xt{dt}")
                    for dt in range(n_dt)]
            for dt in range(n_dt):
                nc.sync.dma_start(wg1ts[dt][:], wg1[dt * 128:(dt + 1) * 128, :])
                nc.sync.dma_start(xts0[dt][:], xT[dt * 128:(dt + 1) * 128, 0:512])
            for ct in range(n_ct):
                if ct == 0:
                    xts = xts0
                else:
                    xts = [xpool.tile([128, 512], F32R, tag=f"xt{dt}", name=f"xt{dt}")
                           for dt in range(n_dt)]
                    for dt in range(n_dt):
                        nc.sync.dma_start(
                            xts[dt][:],
                            xT[dt * 128:(dt + 1) * 128, ct * 512:(ct + 1) * 512])
                for gt in range(n_gt):
                    pg = ps.tile([128, 512], F32, tag="pg")
                    for dt in range(n_dt):
                        nc.tensor.matmul(
                            pg[:],
                            wg1ts[dt][:, gt * 128:(gt + 1) * 128],
                            xts[dt][:],
                            start=(dt == 0), stop=(dt == n_dt - 1),
                        )
                    ot = opool.tile([128, 512], F32, tag="ot")
                    nc.vector.tensor_copy(ot[:], pg[:])
                    nc.sync.dma_start(
                        hT[gt * 128:(gt + 1) * 128, ct * 512:(ct + 1) * 512], ot[:])
    nc.compile()
    return nc


def _build_expert_kernel(C):
    """Per-core: yT[D, C] = (relu(xT.T @ W_in) @ W_out)^T with f32r matmuls."""
    assert C % 128 == 0
    nc = bacc.Bacc("TRN2", target_bir_lowering=False, debug=False)
    xT = nc.dram_tensor("xT", [D, C], F32R, kind="ExternalInput").ap()
    w1 = nc.dram_tensor("w1", [D, H], F32R, kind="ExternalInput").ap()
    w2 = nc.dram_tensor("w2", [H, D], F32R, kind="ExternalInput").ap()
    yT = nc.dram_tensor("yT", [D, C], F32, kind="ExternalOutput").ap()
    n_dt = D // 128
    n_ht = H // 128

    # c-tiles: 512s, keeping any tail >= 256 (f32r needs N >= 256 for 1 cyc/row)
    ctiles = []
    off = 0
    while off < C:
        rem = C - off
        step = 512 if rem >= 512 else rem
        if rem == 640:
            step = 384
        ctiles.append((off, step))
        off += step

    with tile.TileContext(nc) as tc:
        with (
            tc.tile_pool(name="wpool", bufs=1) as wpool,
            tc.tile_pool(name="xpool", bufs=3) as xpool,
            tc.tile_pool(name="apool", bufs=2) as apool,
            tc.tile_pool(name="opool", bufs=4) as opool,
            tc.tile_pool(name="ps1", bufs=4, space="PSUM") as ps1,
            tc.tile_pool(name="ps2", bufs=4, space="PSUM") as ps2,
        ):
            # Per-128-row weight tiles: fine-grained deps let MM1 start after
            # the first 1MB lands instead of after all 8MB of weights.
            def load_x(c0, cw):
                xts = [xpool.tile([128, 512], F32R, tag=f"xt{dt}", name=f"xt{dt}")
                       for dt in range(n_dt)]
                for dt in range(n_dt):
                    nc.sync.dma_start(xts[dt][:, :cw],
                                      xT[dt * 128:(dt + 1) * 128, c0:c0 + cw])
                return xts

            w1ts = [wpool.tile([128, H], F32R, tag=f"w1_{dt}", name=f"w1_{dt}") for dt in range(n_dt)]
            w2ts = [wpool.tile([128, D], F32R, tag=f"w2_{ht}", name=f"w2_{ht}") for ht in range(n_ht)]
            # DMA emission order == consumption order: w1[dt] interleaved with
            # the first c-tile's x, then w2 (needed ~13us later by MM2).
            xts0 = None
            for dt in range(n_dt):
                nc.sync.dma_start(w1ts[dt][:], w1[dt * 128:(dt + 1) * 128, :])
                if dt == 0:
                    xts0 = load_x(*ctiles[0])
            for ht in range(n_ht):
                nc.sync.dma_start(w2ts[ht][:], w2[ht * 128:(ht + 1) * 128, :])

            for ci, (c0, cw) in enumerate(ctiles):
                xts = xts0 if ci == 0 else load_x(c0, cw)
                ats = [apool.tile([128, 512], F32R, tag=f"at{ht}", name=f"at{ht}")
                       for ht in range(n_ht)]
                for ht in range(n_ht):
                    pa = ps1.tile([128, 512], F32, tag="pa")
                    for dt in range(n_dt):
                        nc.tensor.matmul(
                            pa[:, :cw],
                            w1ts[dt][:, ht * 128:(ht + 1) * 128],
                            xts[dt][:, :cw],
                            start=(dt == 0), stop=(dt == n_dt - 1),
                        )
                    nc.scalar.activation(ats[ht][:, :cw], pa[:, :cw], AF.Relu)
                for dt in range(n_dt):
                    py = ps2.tile([128, 512], F32, tag="py")
                    for ht in range(n_ht):
                        nc.tensor.matmul(
                            py[:, :cw],
                            w2ts[ht][:, dt * 128:(dt + 1) * 128],
                            ats[ht][:, :cw],
                            start=(ht == 0), stop=(ht == n_ht - 1),
                        )
                    ot = opool.tile([128, 512], F32, tag="ot")
                    nc.vector.tensor_copy(ot[:, :cw], py[:, :cw])
                    nc.sync.dma_start(yT[dt * 128:(dt + 1) * 128, c0:c0 + cw],
                                      ot[:, :cw])
    nc.compile()
    return nc


def _get_kernel(name, builder, *args):
    key = (name,) + args
    if key not in _kernel_cache:
        _kernel_cache[key] = builder(*args)
    return _kernel_cache[key]


def _run(nc, in_maps, phase):
    res = run_bass_kernel_spmd(nc, in_maps, list(range(NCORES)))
    if res.exec_time_ns is not None:
        last_exec_ns[phase] = res.exec_time_ns
    return res.results


def kernel(x, Wg1, Wg2, W_in, W_out):
    x = np.asarray(x, np.float32)
    Wg1 = np.asarray(Wg1, np.float32)
    Wg2 = np.asarray(Wg2, np.float32)
    W_in = np.asarray(W_in, np.float32)
    W_out = np.asarray(W_out, np.float32)

    xf = x.reshape(N, D)
    xT = np.ascontiguousarray(xf.T)  # [D, N]

    # ---- phase 1: gate hidden (device, exact fp32) ----
    nc_g = _get_kernel("gate", _build_gate_kernel)
    gmaps = [
        {"xT": np.ascontiguousarray(xT[:, i * NSHARD:(i + 1) * NSHARD]),
         "wg1": Wg1}
        for i in range(NCORES)
    ]
    gres = _run(nc_g, gmaps, "gate")
    h_pre = np.concatenate([r["hT"].T for r in gres], axis=0)  # [N, G]

    # ---- host: finish the gate exactly as reference._gate (fp32) ----
    hh = np.tanh(h_pre)
    logits = hh @ Wg2  # [N, E]
    zmax = logits.max(axis=1, keepdims=True)
    ez = np.exp(logits - zmax)
    probs = ez / ez.sum(axis=1, keepdims=True)

    # f32r gate noise (~1e-4 in h) can only flip top-2 picks where the
    # 2nd/3rd probs are within ~1e-3; recompute those tokens' gate exactly.
    sp = np.sort(probs, axis=1)
    border = np.nonzero(sp[:, -2] - sp[:, -3] < 1e-3)[0]
    if len(border):
        hb = np.tanh(xf[border] @ Wg1)
        lb = hb @ Wg2
        eb = np.exp(lb - lb.max(axis=1, keepdims=True))
        probs[border] = eb / eb.sum(axis=1, keepdims=True)

    ar = np.arange(N)
    i1 = probs.argmax(axis=1)
    v1 = probs[ar, i1]
    pm = probs.copy()
    pm[ar, i1] = -np.inf
    i2 = pm.argmax(axis=1)
    v2 = probs[ar, i2]

    # MI aux loss (mirrors _mi_loss in fp32)
    p_mean = probs.mean(axis=0)
    h_mean = -np.sum(p_mean * np.log(p_mean + EPS))
    mean_h = -np.mean(np.sum(probs * np.log(probs + EPS), axis=1))
    loss = np.float32(-h_mean + mean_h)

    # ---- host: all-to-all dispatch ----
    top_i = np.stack([i1, i2], axis=1).ravel()
    top_v = np.stack([v1, v2], axis=1).ravel().astype(np.float32)
    tok = np.repeat(ar, TOP_K)
    order = np.argsort(top_i, kind="stable")
    tok_sorted = tok[order]
    wt_sorted = top_v[order]
    loads = np.bincount(top_i, minlength=E)
    starts = np.concatenate([[0], np.cumsum(loads)])
    C = max(512, -(-int(loads.max()) // 128) * 128)

    emaps = []
    idx_list = []
    for e in range(E):
        idx = tok_sorted[starts[e]:starts[e + 1]]
        wt = wt_sorted[starts[e]:starts[e + 1]]
        idx_list.append(idx)
        buf = np.zeros((D, C), np.float32)
        buf[:, :len(idx)] = xT[:, idx] * wt[None, :]
        emaps.append({"xT": buf, "w1": W_in[e], "w2": W_out[e]})

    # ---- phase 2: expert FFNs (device, f32r) ----
    nc_e = _get_kernel("expert", _build_expert_kernel, C)
    eres = _run(nc_e, emaps, "expert")

    # ---- host: combine ----
    out = np.zeros((N, D), np.float32)
    for e in range(E):
        idx = idx_list[e]
        out[idx] += np.ascontiguousarray(eres[e]["yT"].T[:len(idx)])

    return out.reshape(B, S, D), loss


# revision 6
# speedup vs baseline: 1.0099x; 1.0099x over previous
"""MoE (top-2 of 8 experts, D=H=1024, gate MLP 1024->256->8) on 8 Trainium2
NeuronCores, expert-parallel.

Strategy (per the expert-parallel sharding):
  phase 1 (SPMD, data-parallel): each core computes the gate hidden
      h_pre = x_shard @ Wg1 in exact fp32 (1024 tokens/core).
  host: tanh, logits = h @ Wg2, softmax, top-2 routing, MI aux loss, and the
      all-to-all dispatch: gather each expert's tokens (scaled by their gate
      weight -- valid since relu(g*x W) = g*relu(x W) for g>0).
  phase 2 (SPMD, expert-parallel): core e computes
      yT = (relu(x_e @ W_in[e]) @ W_out[e])^T over its capacity-padded token
      set, with float32r matmuls (1 cyc/row on the PE vs 4 for fp32).
  host: scatter-add combine back to [N, D].
"""

import numpy as np
import concourse.mybir as mybir
import concourse.tile as tile
from concourse import bacc
from concourse.bass_utils import run_bass_kernel_spmd

F32 = mybir.dt.float32
F32R = mybir.dt.float32r
AF = mybir.ActivationFunctionType

TOP_K = 2
EPS = 1e-9
B, S, D, H, E, G = 4, 2048, 1024, 1024, 8, 256
N = B * S
NCORES = 8
NSHARD = N // NCORES

# test.py introspection: phase name -> exec_time_ns of the last kernel() call
# (populated only when BASS_TRACE=1 enables NTFF profiling).
last_exec_ns = {}

_kernel_cache = {}


def _build_gate_kernel():
    """Per-core: hT[256, NSHARD] = (x_shard @ Wg1)^T, f32r matmuls (~1e-4 abs
    noise; borderline top-2 routing decisions are re-verified exactly on host)."""
    nc = bacc.Bacc("TRN2", target_bir_lowering=False, debug=False)
    xT = nc.dram_tensor("xT", [D, NSHARD], F32R, kind="ExternalInput").ap()
    wg1 = nc.dram_tensor("wg1", [D, G], F32R, kind="ExternalInput").ap()
    hT = nc.dram_tensor("hT", [G, NSHARD], F32, kind="ExternalOutput").ap()
    n_ct = NSHARD // 512
    n_dt = D // 128
    n_gt = G // 128
    with tile.TileContext(nc) as tc:
        with (
            tc.tile_pool(name="wpool", bufs=1) as wpool,
            tc.tile_pool(name="xpool", bufs=2) as xpool,
            tc.tile_pool(name="opool", bufs=4) as opool,
            tc.tile_pool(name="ps", bufs=4, space="PSUM") as ps,
        ):
            wg1ts = [wpool.tile([128, G], F32R, tag=f"wg1_{dt}", name=f"wg1_{dt}")
                     for dt in range(n_dt)]
            xts0 = [xpool.tile([128, 512], F32R, tag=f"xt{dt}", name=f"xt{dt}")
                    for dt in range(n_dt)]
            for dt in range(n_dt):
                e1 = nc.sync if dt % 2 == 0 else nc.scalar
                e2 = nc.scalar if dt % 2 == 0 else nc.sync
                e1.dma_start(wg1ts[dt][:], wg1[dt * 128:(dt + 1) * 128, :])
                e2.dma_start(xts0[dt][:], xT[dt * 128:(dt + 1) * 128, 0:512])
            for ct in range(n_ct):
                if ct == 0:
                    xts = xts0
                else:
                    xts = [xpool.tile([128, 512], F32R, tag=f"xt{dt}", name=f"xt{dt}")
                           for dt in range(n_dt)]
                    for dt in range(n_dt):
                        eng = nc.sync if dt % 2 == 0 else nc.scalar
                        eng.dma_start(
                            xts[dt][:],
                            xT[dt * 128:(dt + 1) * 128, ct * 512:(ct + 1) * 512])
                for gt in range(n_gt):
                    pg = ps.tile([128, 512], F32, tag="pg")
                    for dt in range(n_dt):
                        nc.tensor.matmul(
                            pg[:],
                            wg1ts[dt][:, gt * 128:(gt + 1) * 128],
                            xts[dt][:],
                            start=(dt == 0), stop=(dt == n_dt - 1),
                        )
                    ot = opool.tile([128, 512], F32, tag="ot")
                    nc.vector.tensor_copy(ot[:], pg[:])
                    eng = nc.sync if gt % 2 == 0 else nc.scalar
                    eng.dma_start(
                        hT[gt * 128:(gt + 1) * 128, ct * 512:(ct + 1) * 512], ot[:])
    nc.compile()
    return nc


def _build_expert_kernel(C):
    """Per-core: yT[D, C] = (relu(xT.T @ W_in) @ W_out)^T with f32r matmuls."""
    assert C % 128 == 0
    nc = bacc.Bacc("TRN2", target_bir_lowering=False, debug=False)
    xT = nc.dram_tensor("xT", [D, C], F32R, kind="ExternalInput").ap()
    w1 = nc.dram_tensor("w1", [D, H], F32R, kind="ExternalInput").ap()
    w2 = nc.dram_tensor("w2", [H, D], F32R, kind="ExternalInput").ap()
    yT = nc.dram_tensor("yT", [D, C], F32, kind="ExternalOutput").ap()
    n_dt = D // 128
    n_ht = H // 128

    # c-tiles: 512s, keeping any tail >= 256 (f32r needs N >= 256 for 1 cyc/row)
    ctiles = []
    off = 0
    while off < C:
        rem = C - off
        step = 512 if rem >= 512 else rem
        if rem == 640:
            step = 384
        ctiles.append((off, step))
        off += step

    with tile.TileContext(nc) as tc:
        with (
            tc.tile_pool(name="wpool", bufs=1) as wpool,
            tc.tile_pool(name="xpool", bufs=3) as xpool,
            tc.tile_pool(name="apool", bufs=2) as apool,
            tc.tile_pool(name="opool", bufs=4) as opool,
            tc.tile_pool(name="ps1", bufs=4, space="PSUM") as ps1,
            tc.tile_pool(name="ps2", bufs=4, space="PSUM") as ps2,
        ):
            # Per-128-row weight tiles: fine-grained deps let MM1 start after
            # the first 1MB lands instead of after all 8MB of weights.
            def load_x(c0, cw):
                xts = [xpool.tile([128, 512], F32R, tag=f"xt{dt}", name=f"xt{dt}")
                       for dt in range(n_dt)]
                for dt in range(n_dt):
                    eng = nc.sync if dt % 2 == 0 else nc.scalar
                    eng.dma_start(xts[dt][:, :cw],
                                  xT[dt * 128:(dt + 1) * 128, c0:c0 + cw])
                return xts

            w1ts = [wpool.tile([128, H], F32R, tag=f"w1_{dt}", name=f"w1_{dt}") for dt in range(n_dt)]
            w2ts = [wpool.tile([128, D], F32R, tag=f"w2_{ht}", name=f"w2_{ht}") for ht in range(n_ht)]
            # DMA emission order == consumption order: w1[dt] interleaved with
            # the first c-tile's x, then w2 (needed ~13us later by MM2).
            xts0 = None
            for dt in range(n_dt):
                eng = nc.scalar if dt % 2 == 0 else nc.sync
                eng.dma_start(w1ts[dt][:], w1[dt * 128:(dt + 1) * 128, :])
                if dt == 0:
                    xts0 = load_x(*ctiles[0])
            for ht in range(n_ht):
                eng = nc.sync if ht % 2 == 0 else nc.scalar
                eng.dma_start(w2ts[ht][:], w2[ht * 128:(ht + 1) * 128, :])

            for ci, (c0, cw) in enumerate(ctiles):
                xts = xts0 if ci == 0 else load_x(c0, cw)
                ats = [apool.tile([128, 512], F32R, tag=f"at{ht}", name=f"at{ht}")
                       for ht in range(n_ht)]
                for ht in range(n_ht):
                    pa = ps1.tile([128, 512], F32, tag="pa")
                    for dt in range(n_dt):
                        nc.tensor.matmul(
                            pa[:, :cw],
                            w1ts[dt][:, ht * 128:(ht + 1) * 128],
                            xts[dt][:, :cw],
                            start=(dt == 0), stop=(dt == n_dt - 1),
                        )
                    nc.scalar.activation(ats[ht][:, :cw], pa[:, :cw], AF.Relu)
                for dt in range(n_dt):
                    py = ps2.tile([128, 512], F32, tag="py")
                    for ht in range(n_ht):
                        nc.tensor.matmul(
                            py[:, :cw],
                            w2ts[ht][:, dt * 128:(dt + 1) * 128],
                            ats[ht][:, :cw],
                            start=(ht == 0), stop=(ht == n_ht - 1),
                        )
                    ot = opool.tile([128, 512], F32, tag="ot")
                    nc.vector.tensor_copy(ot[:, :cw], py[:, :cw])
                    eng = nc.sync if dt % 2 == 0 else nc.scalar
                    eng.dma_start(yT[dt * 128:(dt + 1) * 128, c0:c0 + cw],
                                  ot[:, :cw])
    nc.compile()
    return nc


def _get_kernel(name, builder, *args):
    key = (name,) + args
    if key not in _kernel_cache:
        _kernel_cache[key] = builder(*args)
    return _kernel_cache[key]


def _run(nc, in_maps, phase):
    res = run_bass_kernel_spmd(nc, in_maps, list(range(NCORES)))
    if res.exec_time_ns is not None:
        last_exec_ns[phase] = res.exec_time_ns
    return res.results


def kernel(x, Wg1, Wg2, W_in, W_out):
    x = np.asarray(x, np.float32)
    Wg1 = np.asarray(Wg1, np.float32)
    Wg2 = np.asarray(Wg2, np.float32)
    W_in = np.asarray(W_in, np.float32)
    W_out = np.asarray(W_out, np.float32)

    xf = x.reshape(N, D)
    xT = np.ascontiguousarray(xf.T)  # [D, N]

    # ---- phase 1: gate hidden (device, exact fp32) ----
    nc_g = _get_kernel("gate", _build_gate_kernel)
    gmaps = [
        {"xT": np.ascontiguousarray(xT[:, i * NSHARD:(i + 1) * NSHARD]),
         "wg1": Wg1}
        for i in range(NCORES)
    ]
    gres = _run(nc_g, gmaps, "gate")
    h_pre = np.concatenate([r["hT"].T for r in gres], axis=0)  # [N, G]

    # ---- host: finish the gate exactly as reference._gate (fp32) ----
    hh = np.tanh(h_pre)
    logits = hh @ Wg2  # [N, E]
    zmax = logits.max(axis=1, keepdims=True)
    ez = np.exp(logits - zmax)
    probs = ez / ez.sum(axis=1, keepdims=True)

    # f32r gate noise (~1e-4 in h) can only flip top-2 picks where the
    # 2nd/3rd probs are within ~1e-3; recompute those tokens' gate exactly.
    sp = np.sort(probs, axis=1)
    border = np.nonzero(sp[:, -2] - sp[:, -3] < 1e-3)[0]
    if len(border):
        hb = np.tanh(xf[border] @ Wg1)
        lb = hb @ Wg2
        eb = np.exp(lb - lb.max(axis=1, keepdims=True))
        probs[border] = eb / eb.sum(axis=1, keepdims=True)

    ar = np.arange(N)
    i1 = probs.argmax(axis=1)
    v1 = probs[ar, i1]
    pm = probs.copy()
    pm[ar, i1] = -np.inf
    i2 = pm.argmax(axis=1)
    v2 = probs[ar, i2]

    # MI aux loss (mirrors _mi_loss in fp32)
    p_mean = probs.mean(axis=0)
    h_mean = -np.sum(p_mean * np.log(p_mean + EPS))
    mean_h = -np.mean(np.sum(probs * np.log(probs + EPS), axis=1))
    loss = np.float32(-h_mean + mean_h)

    # ---- host: all-to-all dispatch ----
    top_i = np.stack([i1, i2], axis=1).ravel()
    top_v = np.stack([v1, v2], axis=1).ravel().astype(np.float32)
    tok = np.repeat(ar, TOP_K)
    order = np.argsort(top_i, kind="stable")
    tok_sorted = tok[order]
    wt_sorted = top_v[order]
    loads = np.bincount(top_i, minlength=E)
    starts = np.concatenate([[0], np.cumsum(loads)])
    C = max(512, -(-int(loads.max()) // 128) * 128)

    emaps = []
    idx_list = []
    for e in range(E):
        idx = tok_sorted[starts[e]:starts[e + 1]]
        wt = wt_sorted[starts[e]:starts[e + 1]]
        idx_list.append(idx)
        buf = np.zeros((D, C), np.float32)
        buf[:, :len(idx)] = xT[:, idx] * wt[None, :]
        emaps.append({"xT": buf, "w1": W_in[e], "w2": W_out[e]})

    # ---- phase 2: expert FFNs (device, f32r) ----
    nc_e = _get_kernel("expert", _build_expert_kernel, C)
    eres = _run(nc_e, emaps, "expert")

    # ---- host: combine ----
    out = np.zeros((N, D), np.float32)
    for e in range(E):
        idx = idx_list[e]
        out[idx] += np.ascontiguousarray(eres[e]["yT"].T[:len(idx)])

    return out.reshape(B, S, D), loss


# revision 7
# speedup vs baseline: 1.0631x; 1.0527x over previous
"""MoE (top-2 of 8 experts, D=H=1024, gate MLP 1024->256->8) on 8 Trainium2
NeuronCores, expert-parallel.

Strategy (per the expert-parallel sharding):
  phase 1 (SPMD, data-parallel): each core computes the gate hidden
      h_pre = x_shard @ Wg1 in exact fp32 (1024 tokens/core).
  host: tanh, logits = h @ Wg2, softmax, top-2 routing, MI aux loss, and the
      all-to-all dispatch: gather each expert's tokens (scaled by their gate
      weight -- valid since relu(g*x W) = g*relu(x W) for g>0).
  phase 2 (SPMD, expert-parallel): core e computes
      yT = (relu(x_e @ W_in[e]) @ W_out[e])^T over its capacity-padded token
      set, with float32r matmuls (1 cyc/row on the PE vs 4 for fp32).
  host: scatter-add combine back to [N, D].
"""

import numpy as np
import concourse.mybir as mybir
import concourse.tile as tile
from concourse import bacc
from concourse.bass_utils import run_bass_kernel_spmd

F32 = mybir.dt.float32
F32R = mybir.dt.float32r
AF = mybir.ActivationFunctionType

TOP_K = 2
EPS = 1e-9
B, S, D, H, E, G = 4, 2048, 1024, 1024, 8, 256
N = B * S
NCORES = 8
NSHARD = N // NCORES

# test.py introspection: phase name -> exec_time_ns of the last kernel() call
# (populated only when BASS_TRACE=1 enables NTFF profiling).
last_exec_ns = {}

_kernel_cache = {}


def _build_gate_kernel():
    """Per-core: hT[256, NSHARD] = (x_shard @ Wg1)^T, f32r matmuls (~1e-4 abs
    noise; borderline top-2 routing decisions are re-verified exactly on host)."""
    nc = bacc.Bacc("TRN2", target_bir_lowering=False, debug=False)
    xT = nc.dram_tensor("xT", [D, NSHARD], F32R, kind="ExternalInput").ap()
    wg1 = nc.dram_tensor("wg1", [D, G], F32R, kind="ExternalInput").ap()
    hT = nc.dram_tensor("hT", [G, NSHARD], F32, kind="ExternalOutput").ap()
    n_ct = NSHARD // 512
    n_dt = D // 128
    n_gt = G // 128
    with tile.TileContext(nc) as tc:
        with (
            tc.tile_pool(name="wpool", bufs=1) as wpool,
            tc.tile_pool(name="xpool", bufs=2) as xpool,
            tc.tile_pool(name="opool", bufs=4) as opool,
            tc.tile_pool(name="ps", bufs=4, space="PSUM") as ps,
        ):
            wg1ts = [wpool.tile([128, G], F32R, tag=f"wg1_{dt}", name=f"wg1_{dt}")
                     for dt in range(n_dt)]
            xts0 = [xpool.tile([128, 512], F32R, tag=f"xt{dt}", name=f"xt{dt}")
                    for dt in range(n_dt)]
            for dt in range(n_dt):
                e1 = nc.sync if dt % 2 == 0 else nc.scalar
                e2 = nc.scalar if dt % 2 == 0 else nc.sync
                e1.dma_start(wg1ts[dt][:], wg1[dt * 128:(dt + 1) * 128, :])
                e2.dma_start(xts0[dt][:], xT[dt * 128:(dt + 1) * 128, 0:512])
            for ct in range(n_ct):
                if ct == 0:
                    xts = xts0
                else:
                    xts = [xpool.tile([128, 512], F32R, tag=f"xt{dt}", name=f"xt{dt}")
                           for dt in range(n_dt)]
                    for dt in range(n_dt):
                        eng = nc.sync if dt % 2 == 0 else nc.scalar
                        eng.dma_start(
                            xts[dt][:],
                            xT[dt * 128:(dt + 1) * 128, ct * 512:(ct + 1) * 512])
                for gt in range(n_gt):
                    pg = ps.tile([128, 512], F32, tag="pg")
                    for dt in range(n_dt):
                        nc.tensor.matmul(
                            pg[:],
                            wg1ts[dt][:, gt * 128:(gt + 1) * 128],
                            xts[dt][:],
                            start=(dt == 0), stop=(dt == n_dt - 1),
                        )
                    ot = opool.tile([128, 512], F32, tag="ot")
                    nc.vector.tensor_copy(ot[:], pg[:])
                    eng = nc.sync if gt % 2 == 0 else nc.scalar
                    eng.dma_start(
                        hT[gt * 128:(gt + 1) * 128, ct * 512:(ct + 1) * 512], ot[:])
    nc.compile()
    return nc


def _build_expert_kernel(C):
    """Per-core: yT[D, C] = (relu(xT.T @ W_in) @ W_out)^T with f32r matmuls."""
    assert C % 128 == 0
    nc = bacc.Bacc("TRN2", target_bir_lowering=False, debug=False)
    xT = nc.dram_tensor("xT", [D, C], F32R, kind="ExternalInput").ap()
    w1 = nc.dram_tensor("w1", [D, H], F32R, kind="ExternalInput").ap()
    w2 = nc.dram_tensor("w2", [H, D], F32R, kind="ExternalInput").ap()
    yT = nc.dram_tensor("yT", [D, C], F32, kind="ExternalOutput").ap()
    n_dt = D // 128
    n_ht = H // 128

    # c-tiles: 512s, keeping any tail >= 256 (f32r needs N >= 256 for 1 cyc/row)
    ctiles = []
    off = 0
    while off < C:
        rem = C - off
        step = 512 if rem >= 512 else rem
        if rem == 640:
            step = 384
        ctiles.append((off, step))
        off += step

    with tile.TileContext(nc) as tc:
        with (
            tc.tile_pool(name="wpool", bufs=1) as wpool,
            tc.tile_pool(name="xpool", bufs=3) as xpool,
            tc.tile_pool(name="apool", bufs=2) as apool,
            tc.tile_pool(name="opool", bufs=4) as opool,
            tc.tile_pool(name="ps1", bufs=4, space="PSUM") as ps1,
            tc.tile_pool(name="ps2", bufs=4, space="PSUM") as ps2,
        ):
            # Per-128-row weight tiles: fine-grained deps let MM1 start after
            # the first 1MB lands instead of after all 8MB of weights.
            def load_x(c0, cw, head=False):
                xts = [xpool.tile([128, 512], F32R, tag=f"xt{dt}", name=f"xt{dt}")
                       for dt in range(n_dt)]
                for dt in range(n_dt):
                    # scalar-engine DMA only at the head: later, ACT's strict
                    # FIFO would park these behind relus and stall the PE
                    eng = nc.scalar if head and dt % 2 else nc.sync
                    eng.dma_start(xts[dt][:, :cw],
                                  xT[dt * 128:(dt + 1) * 128, c0:c0 + cw])
                return xts

            w1ts = [wpool.tile([128, H], F32R, tag=f"w1_{dt}", name=f"w1_{dt}") for dt in range(n_dt)]
            w2ts = [wpool.tile([128, D], F32R, tag=f"w2_{ht}", name=f"w2_{ht}") for ht in range(n_ht)]
            # DMA emission order == consumption order: w1[dt] interleaved with
            # the first c-tile's x, then w2 (needed ~13us later by MM2).
            xts0 = None
            for dt in range(n_dt):
                eng = nc.scalar if dt % 2 == 0 else nc.sync
                eng.dma_start(w1ts[dt][:], w1[dt * 128:(dt + 1) * 128, :])
                if dt == 0:
                    xts0 = load_x(*ctiles[0], head=True)
            for ht in range(n_ht):
                eng = nc.sync if ht % 2 == 0 else nc.scalar
                eng.dma_start(w2ts[ht][:], w2[ht * 128:(ht + 1) * 128, :])

            for ci, (c0, cw) in enumerate(ctiles):
                xts = xts0 if ci == 0 else load_x(c0, cw)
                ats = [apool.tile([128, 512], F32R, tag=f"at{ht}", name=f"at{ht}")
                       for ht in range(n_ht)]
                for ht in range(n_ht):
                    pa = ps1.tile([128, 512], F32, tag="pa")
                    for dt in range(n_dt):
                        nc.tensor.matmul(
                            pa[:, :cw],
                            w1ts[dt][:, ht * 128:(ht + 1) * 128],
                            xts[dt][:, :cw],
                            start=(dt == 0), stop=(dt == n_dt - 1),
                        )
                    nc.scalar.activation(ats[ht][:, :cw], pa[:, :cw], AF.Relu)
                for dt in range(n_dt):
                    py = ps2.tile([128, 512], F32, tag="py")
                    for ht in range(n_ht):
                        nc.tensor.matmul(
                            py[:, :cw],
                            w2ts[ht][:, dt * 128:(dt + 1) * 128],
                            ats[ht][:, :cw],
                            start=(ht == 0), stop=(ht == n_ht - 1),
                        )
                    ot = opool.tile([128, 512], F32, tag="ot")
                    nc.vector.tensor_copy(ot[:, :cw], py[:, :cw])
                    nc.sync.dma_start(yT[dt * 128:(dt + 1) * 128, c0:c0 + cw],
                                      ot[:, :cw])
    nc.compile()
    return nc


def _get_kernel(name, builder, *args):
    key = (name,) + args
    if key not in _kernel_cache:
        _kernel_cache[key] = builder(*args)
    return _kernel_cache[key]


def _run(nc, in_maps, phase):
    res = run_bass_kernel_spmd(nc, in_maps, list(range(NCORES)))
    if res.exec_time_ns is not None:
        last_exec_ns[phase] = res.exec_time_ns
    return res.results


def kernel(x, Wg1, Wg2, W_in, W_out):
    x = np.asarray(x, np.float32)
    Wg1 = np.asarray(Wg1, np.float32)
    Wg2 = np.asarray(Wg2, np.float32)
    W_in = np.asarray(W_in, np.float32)
    W_out = np.asarray(W_out, np.float32)

    xf = x.reshape(N, D)
    xT = np.ascontiguousarray(xf.T)  # [D, N]

    # ---- phase 1: gate hidden (device, exact fp32) ----
    nc_g = _get_kernel("gate", _build_gate_kernel)
    gmaps = [
        {"xT": np.ascontiguousarray(xT[:, i * NSHARD:(i + 1) * NSHARD]),
         "wg1": Wg1}
        for i in range(NCORES)
    ]
    gres = _run(nc_g, gmaps, "gate")
    h_pre = np.concatenate([r["hT"].T for r in gres], axis=0)  # [N, G]

    # ---- host: finish the gate exactly as reference._gate (fp32) ----
    hh = np.tanh(h_pre)
    logits = hh @ Wg2  # [N, E]
    zmax = logits.max(axis=1, keepdims=True)
    ez = np.exp(logits - zmax)
    probs = ez / ez.sum(axis=1, keepdims=True)

    # f32r gate noise (~1e-4 in h) can only flip top-2 picks where the
    # 2nd/3rd probs are within ~1e-3; recompute those tokens' gate exactly.
    sp = np.sort(probs, axis=1)
    border = np.nonzero(sp[:, -2] - sp[:, -3] < 1e-3)[0]
    if len(border):
        hb = np.tanh(xf[border] @ Wg1)
        lb = hb @ Wg2
        eb = np.exp(lb - lb.max(axis=1, keepdims=True))
        probs[border] = eb / eb.sum(axis=1, keepdims=True)

    ar = np.arange(N)
    i1 = probs.argmax(axis=1)
    v1 = probs[ar, i1]
    pm = probs.copy()
    pm[ar, i1] = -np.inf
    i2 = pm.argmax(axis=1)
    v2 = probs[ar, i2]

    # MI aux loss (mirrors _mi_loss in fp32)
    p_mean = probs.mean(axis=0)
    h_mean = -np.sum(p_mean * np.log(p_mean + EPS))
    mean_h = -np.mean(np.sum(probs * np.log(probs + EPS), axis=1))
    loss = np.float32(-h_mean + mean_h)

    # ---- host: all-to-all dispatch ----
    top_i = np.stack([i1, i2], axis=1).ravel()
    top_v = np.stack([v1, v2], axis=1).ravel().astype(np.float32)
    tok = np.repeat(ar, TOP_K)
    order = np.argsort(top_i, kind="stable")
    tok_sorted = tok[order]
    wt_sorted = top_v[order]
    loads = np.bincount(top_i, minlength=E)
    starts = np.concatenate([[0], np.cumsum(loads)])
    C = max(512, -(-int(loads.max()) // 128) * 128)

    emaps = []
    idx_list = []
    for e in range(E):
        idx = tok_sorted[starts[e]:starts[e + 1]]
        wt = wt_sorted[starts[e]:starts[e + 1]]
        idx_list.append(idx)
        buf = np.zeros((D, C), np.float32)
        buf[:, :len(idx)] = xT[:, idx] * wt[None, :]
        emaps.append({"xT": buf, "w1": W_in[e], "w2": W_out[e]})

    # ---- phase 2: expert FFNs (device, f32r) ----
    nc_e = _get_kernel("expert", _build_expert_kernel, C)
    eres = _run(nc_e, emaps, "expert")

    # ---- host: combine ----
    out = np.zeros((N, D), np.float32)
    for e in range(E):
        idx = idx_list[e]
        out[idx] += np.ascontiguousarray(eres[e]["yT"].T[:len(idx)])

    return out.reshape(B, S, D), loss
